# revision 9
# baseline (speedup 1.0000x reference)
"""Trainium2 Bass kernel for nn_Loss_39341900431615.

Reference semantics (B,C,H,W = 16,128,128,128; only tensor[0] is read):
    idx = argmax(tensor[0,0].reshape(-1))        # row-major first max
    x0, y0 = idx // W, idx % W
    wgt[j,k] = (x0-j)^2 + (y0-k)^2               # [H,W] = [128,128]
    out[w] = sum_{j,k} wgt[j,k] * tensor[0,j,k,w]  # [W] = [128]

Sharding: the j axis (channel dim of tensor[0]) is split across 8 cores,
16 j-planes each (1 MB/core). The [128,128] argmax map tensor[0,0] is
replicated to every core (64 KB, packed in the const blob) and each core
computes the argmax redundantly. Each core emits a [128] partial; the
host sums the 8 partials (the all-reduce is 512 B/core, cheaper on host
than a device collective).

Hardware constraint that shapes the code: this walrus build allows ONE
sync wait per compute instruction (only Drains may carry several). So:
 - all constants ride in a single blob tensor -> one DMA semaphore that
   each engine observes once, via an instruction with no other deps
   (DVE: the map row-max; PE: the iota-row transpose);
 - a dummy PE matmul consumes the slice DMA before the accumulation
   loop so the first real matmul only waits on the DVE weight semaphore.

Per-core device program:
  1. DMA in: blob [128, 273] (map | identity | jvals | iota), slice
     viewed [k=128, j=16, w=128].
  2. Row max + row argmax of the map via DVE max_with_indices.
  3. PE-transposes (against the identity): iota row, rowmax, rowargmax.
     max_with_indices on the transposed rowmax gives gmax and x0 (first
     occurrence == row-major semantics); y0 = sum_r (r==x0)*rowargmax[r].
  4. Broadcast (x0,y0) to all partitions with a K=1 matmul against ones.
  5. wgt_T[k,j] = (jvals[j]-x0)^2 + (iota[k]-y0)^2 via DVE ops.
  6. 16 accumulating PE matmuls: psum[w,1] += slice_j[k,w].T @ wgt_T[:,j:j+1].
  7. Copy PSUM -> SBUF via ACT, DMA out [128,1].
"""

import sys

for _p in ("/opt/trn_rl_repo", "/opt/pypackages"):
    if _p not in sys.path:
        sys.path.insert(0, _p)

import numpy as np

import concourse.bass as bass
from concourse import bacc
import concourse.tile as tile
from concourse import mybir
from concourse.bass_utils import run_bass_kernel_spmd

B, C, H, W = 16, 128, 128, 128
NCORES = 8
JPER = C // NCORES  # 16 j-planes per core
F32 = mybir.dt.float32

# const blob column layout
COL_MAP = 0       # [128, 128] argmax map
COL_ID = 128      # [128, 128] identity
COL_JV = 256      # [128, 16] per-core j values (row-replicated)
COL_IOTA = 272    # [128, 1] partition index 0..127
NCOLS = 273

_CACHE = {}


def _build_bass():
    nc = bacc.Bacc("TRN2", target_bir_lowering=False, debug=False,
                   num_devices=NCORES)

    blob_d = nc.dram_tensor("blob", [128, NCOLS], F32, kind="ExternalInput")
    tslice = nc.dram_tensor("tslice", [JPER, H, W], F32, kind="ExternalInput")
    outd = nc.dram_tensor("out", [W, 1], F32, kind="ExternalOutput")

    with tile.TileContext(nc) as tc:
        with (
            tc.tile_pool(name="main", bufs=1) as pool,
            tc.tile_pool(name="psum", bufs=1, space="PSUM") as psum_pool,
        ):
            blob = pool.tile([128, NCOLS], F32)
            st = pool.tile([128, JPER, W], F32)    # slice as [k, j, w]

            nc.sync.dma_start(out=blob[:, :], in_=blob_d[:, :])
            nc.sync.dma_start(out=st[:, :, :],
                              in_=tslice.ap().rearrange("j k w -> k j w"))

            sm = blob[:, COL_MAP:COL_MAP + 128]    # argmax map [x, y]
            sid = blob[:, COL_ID:COL_ID + 128]     # identity
            sj = blob[:, COL_JV:COL_JV + JPER]     # j values
            si = blob[:, COL_IOTA:COL_IOTA + 1]    # partition index

            # --- per-row max and argmax of the map (DVE observes blob) ---
            vmax8 = pool.tile([128, 8], F32)
            vidx8 = pool.tile([128, 8], mybir.dt.uint32)
            nc.vector.max_with_indices(vmax8, vidx8, sm)

            vidxf = pool.tile([128, 1], F32)       # rowargmax as f32
            nc.vector.tensor_copy(vidxf, vidx8[:, 0:1])

            # --- PE transposes; the iota transpose goes first so PE's
            # only blob-DMA wait lands on an instruction with no DVE dep.
            iotaT_ps = psum_pool.tile([1, 128], F32)
            nc.tensor.transpose(iotaT_ps[:, :], si, sid)
            vmaxT_ps = psum_pool.tile([1, 128], F32)
            nc.tensor.transpose(vmaxT_ps[:, :], vmax8[:, 0:1], sid)
            vidxT_ps = psum_pool.tile([1, 128], F32)
            nc.tensor.transpose(vidxT_ps[:, :], vidxf[:, :], sid)

            iotaw = pool.tile([1, 128], F32)
            nc.vector.tensor_copy(iotaw, iotaT_ps[:, :])
            vmaxT = pool.tile([1, 128], F32)
            nc.vector.tensor_copy(vmaxT, vmaxT_ps[:, :])
            vidxT = pool.tile([1, 128], F32)
            nc.vector.tensor_copy(vidxT, vidxT_ps[:, :])

            # global max over rows: value + first row index (= x0)
            gv8 = pool.tile([1, 8], F32)
            gi8 = pool.tile([1, 8], mybir.dt.uint32)
            nc.vector.max_with_indices(gv8, gi8, vmaxT[:, :])

            xs = pool.tile([1, 2], F32)            # (x0, y0) on partition 0
            nc.vector.tensor_copy(xs[:, 0:1], gi8[:, 0:1])

            # y0 = sum_r (r == x0) * rowargmax[r]
            ymask = pool.tile([1, 128], F32)
            y0 = pool.tile([1, 1], F32)
            nc.vector.scalar_tensor_tensor(
                ymask, in0=iotaw[:, :], scalar=xs[:, 0:1], in1=vidxT[:, :],
                op0=mybir.AluOpType.is_equal, op1=mybir.AluOpType.mult,
                accum_out=y0,
            )
            nc.vector.tensor_copy(xs[:, 1:2], y0)

            # broadcast (x0, y0) to all partitions via K=1 matmul with ones
            ones = pool.tile([1, 128], F32)
            nc.vector.memset(ones, 1.0)
            xy_ps = psum_pool.tile([128, 2], F32)
            nc.tensor.matmul(xy_ps[:, :], ones[:, :], xs[:, :],
                             start=True, stop=True)
            xy = pool.tile([128, 2], F32)
            nc.vector.tensor_copy(xy, xy_ps[:, :])

            # --- wgt_T[k, j] = (jvals[j]-x0)^2 + (iota[k]-y0)^2 ---
            dj = pool.tile([128, JPER], F32)
            nc.vector.tensor_scalar(dj, sj, xy[:, 0:1], None,
                                    op0=mybir.AluOpType.subtract)
            rowsq = pool.tile([128, JPER], F32)
            nc.vector.tensor_tensor(rowsq, dj, dj, op=mybir.AluOpType.mult)

            dk = pool.tile([128, 1], F32)
            nc.vector.tensor_scalar(dk, si, xy[:, 1:2], None,
                                    op0=mybir.AluOpType.subtract)
            colsq = pool.tile([128, 1], F32)
            nc.vector.tensor_tensor(colsq, dk, dk, op=mybir.AluOpType.mult)

            wgt = pool.tile([128, JPER], F32)
            nc.vector.tensor_scalar(wgt, rowsq, colsq[:, 0:1], None,
                                    op0=mybir.AluOpType.add)

            # Dummy PE consumer of st: takes the st-DMA wait so the first
            # accumulating matmul only needs the DVE (wgt) wait.
            dummy_ps = psum_pool.tile([1, 1], F32)
            nc.tensor.matmul(dummy_ps[:, :], st[:, 0, 0:1], st[:, 0, 0:1],
                             start=True, stop=True)

            # --- out[w] = sum_j slice_j[k,w].T @ wgt[:, j] ---
            acc = psum_pool.tile([128, 1], F32)
            for j in range(JPER):
                nc.tensor.matmul(acc[:, :], st[:, j, :], wgt[:, j:j + 1],
                                 start=(j == 0), stop=(j == JPER - 1))

            outv = pool.tile([128, 1], F32)
            nc.scalar.copy(outv, acc[:, :])
            nc.sync.dma_start(out=outd[:, :], in_=outv[:, :])

    return nc


def _get_bass():
    if "nc" not in _CACHE:
        nc = _build_bass()
        nc.finalize()
        _CACHE["nc"] = nc
    return _CACHE["nc"]


def _make_blob(jlo):
    blob = np.zeros((128, NCOLS), dtype=np.float32)
    blob[:, COL_ID:COL_ID + 128] = np.eye(128, dtype=np.float32)
    blob[:, COL_JV:COL_JV + JPER] = np.arange(jlo, jlo + JPER,
                                              dtype=np.float32)[None, :]
    blob[:, COL_IOTA] = np.arange(128, dtype=np.float32)
    return blob


def _make_in_maps(tensor):
    t0 = np.ascontiguousarray(tensor[0], dtype=np.float32)  # [C,H,W]
    in_maps = []
    for c in range(NCORES):
        jlo = c * JPER
        blob = _make_blob(jlo)
        blob[:, COL_MAP:COL_MAP + 128] = t0[0]
        in_maps.append({
            "blob": blob,
            "tslice": np.ascontiguousarray(t0[jlo:jlo + JPER]),
        })
    return in_maps


def kernel(tensor):
    nc = _get_bass()
    res = run_bass_kernel_spmd(nc, _make_in_maps(tensor),
                               core_ids=list(range(NCORES)))
    partials = np.stack([r["out"].reshape(W) for r in res.results])
    return partials.astype(np.float64).sum(axis=0).astype(np.float32)


# revision 10
# speedup vs baseline: 1.0568x; 1.0568x over previous
"""Trainium2 Bass kernel for nn_Loss_39341900431615.

Reference semantics (B,C,H,W = 16,128,128,128; only tensor[0] is read):
    idx = argmax(tensor[0,0].reshape(-1))        # row-major first max
    x0, y0 = idx // W, idx % W
    wgt[j,k] = (x0-j)^2 + (y0-k)^2               # [H,W] = [128,128]
    out[w] = sum_{j,k} wgt[j,k] * tensor[0,j,k,w]  # [W] = [128]

Sharding: the j axis (channel dim of tensor[0]) is split across 8 cores,
16 j-planes each (1 MB/core). The [128,128] argmax map tensor[0,0] is
replicated to every core (packed in the const blob) and each core
computes the argmax redundantly. Each core emits a [128] partial; the
host sums the 8 partials.

Layout: the per-core slice rides as a flat [128, 2048] tensor — SBUF
partition p holds source plane (j = p//8, k in [16*(p%8), 16*(p%8)+16))
as 8 KB contiguous rows, so the DMA runs at full HBM bandwidth (the
naive [k, j, w] transpose layout ran at ~80 GB/s on 512 B chunks).
The contraction then runs as 16 accumulating PE matmuls over klo:
    psum[w, 1] += st2[:, klo, :].T @ wgt2[:, klo:klo+1]
with wgt2[p, klo] = (jv2[p]-x0)^2 + (kv2[p,klo]-y0)^2 from host tables.

Per-core device program:
  1. DMA in: blob [128, 274] (map | identity | iota | jv2 | kv2),
     slice [128, 2048].
  2. Row max + row argmax of the map via DVE max_with_indices.
  3. PE-transposes (against the identity): iota row, rowmax, rowargmax.
     max_with_indices on the transposed rowmax gives gmax and x0 (first
     occurrence == row-major semantics); y0 = sum_r (r==x0)*rowargmax[r].
  4. Broadcast (x0,y0) to all partitions with a K=1 matmul against ones.
  5. wgt2 via DVE tensor_scalar ops.
  6. 16 accumulating PE matmuls -> psum [128, 1].
  7. Copy PSUM -> SBUF (DVE), DMA out [128, 1].
"""

import sys

for _p in ("/opt/trn_rl_repo", "/opt/pypackages"):
    if _p not in sys.path:
        sys.path.insert(0, _p)

import numpy as np

import concourse.bass as bass
from concourse import bacc
import concourse.tile as tile
from concourse import mybir
from concourse.bass_utils import run_bass_kernel_spmd

B, C, H, W = 16, 128, 128, 128
NCORES = 8
JPER = C // NCORES   # 16 j-planes per core
KLO = 16             # inner contraction steps (k per partition)
KHI = 8              # k groups per partition dim
F32 = mybir.dt.float32

# const blob column layout
COL_MAP = 0            # [128, 128] argmax map
COL_ID = 128           # [128, 128] identity
COL_IOTA = 256         # [128, 1] partition index 0..127
COL_JV = 257           # [128, 1] j(p) = jlo + p//8
COL_KV = 258           # [128, 16] k(p, klo) = (p%8)*16 + klo
NCOLS = 274

_CACHE = {}


def _build_bass():
    nc = bacc.Bacc("TRN2", target_bir_lowering=False, debug=False,
                   num_devices=NCORES, enable_partition_id=False)

    blob_d = nc.dram_tensor("blob", [128, NCOLS], F32, kind="ExternalInput")
    tslice = nc.dram_tensor("tslice", [128, KLO * W], F32,
                            kind="ExternalInput")
    outd = nc.dram_tensor("out", [W, 1], F32, kind="ExternalOutput")

    with tile.TileContext(nc) as tc:
        with (
            tc.tile_pool(name="main", bufs=1) as pool,
            tc.tile_pool(name="psum", bufs=1, space="PSUM") as psum_pool,
        ):
            blob = pool.tile([128, NCOLS], F32)
            st = pool.tile([128, KLO, W], F32)     # [(j,khi), klo, w]

            nc.sync.dma_start(out=blob[:, :], in_=blob_d[:, :])
            nc.sync.dma_start(
                out=st[:, :, :],
                in_=tslice.ap().rearrange("p (a b) -> p a b", a=KLO))

            sm = blob[:, COL_MAP:COL_MAP + 128]    # argmax map [x, y]
            sid = blob[:, COL_ID:COL_ID + 128]     # identity
            si = blob[:, COL_IOTA:COL_IOTA + 1]    # partition index
            jv = blob[:, COL_JV:COL_JV + 1]        # j(p)
            kv = blob[:, COL_KV:COL_KV + KLO]      # k(p, klo)

            # --- per-row max and argmax of the map ---
            vmax8 = pool.tile([128, 8], F32)
            vidx8 = pool.tile([128, 8], mybir.dt.uint32)
            nc.vector.max_with_indices(vmax8, vidx8, sm)

            vidxf = pool.tile([128, 1], F32)       # rowargmax as f32
            nc.vector.tensor_copy(vidxf, vidx8[:, 0:1])

            # --- PE transposes (iota first: its only dep is the blob DMA)
            iotaT_ps = psum_pool.tile([1, 128], F32)
            nc.tensor.transpose(iotaT_ps[:, :], si, sid)
            vmaxT_ps = psum_pool.tile([1, 128], F32)
            nc.tensor.transpose(vmaxT_ps[:, :], vmax8[:, 0:1], sid)
            vidxT_ps = psum_pool.tile([1, 128], F32)
            nc.tensor.transpose(vidxT_ps[:, :], vidxf[:, :], sid)

            iotaw = pool.tile([1, 128], F32)
            nc.vector.tensor_copy(iotaw, iotaT_ps[:, :])
            vmaxT = pool.tile([1, 128], F32)
            nc.vector.tensor_copy(vmaxT, vmaxT_ps[:, :])
            vidxT = pool.tile([1, 128], F32)
            nc.vector.tensor_copy(vidxT, vidxT_ps[:, :])

            # global max over rows: value + first row index (= x0)
            gv8 = pool.tile([1, 8], F32)
            gi8 = pool.tile([1, 8], mybir.dt.uint32)
            nc.vector.max_with_indices(gv8, gi8, vmaxT[:, :])

            xs = pool.tile([1, 2], F32)            # (x0, y0) on partition 0
            nc.vector.tensor_copy(xs[:, 0:1], gi8[:, 0:1])

            # y0 = sum_r (r == x0) * rowargmax[r]
            ymask = pool.tile([1, 128], F32)
            y0 = pool.tile([1, 1], F32)
            nc.vector.scalar_tensor_tensor(
                ymask, in0=iotaw[:, :], scalar=xs[:, 0:1], in1=vidxT[:, :],
                op0=mybir.AluOpType.is_equal, op1=mybir.AluOpType.mult,
                accum_out=y0,
            )
            nc.vector.tensor_copy(xs[:, 1:2], y0)

            # broadcast (x0, y0) to all partitions via K=1 matmul with ones
            ones = pool.tile([1, 128], F32)
            nc.vector.memset(ones, 1.0)
            xy_ps = psum_pool.tile([128, 2], F32)
            nc.tensor.matmul(xy_ps[:, :], ones[:, :], xs[:, :],
                             start=True, stop=True)
            xy = pool.tile([128, 2], F32)
            nc.vector.tensor_copy(xy, xy_ps[:, :])

            # --- wgt2[p, klo] = (jv[p]-x0)^2 + (kv[p,klo]-y0)^2 ---
            d1 = pool.tile([128, 1], F32)
            nc.vector.tensor_scalar(d1, jv, xy[:, 0:1], None,
                                    op0=mybir.AluOpType.subtract)
            sq1 = pool.tile([128, 1], F32)
            nc.vector.tensor_tensor(sq1, d1, d1, op=mybir.AluOpType.mult)

            d2 = pool.tile([128, KLO], F32)
            nc.vector.tensor_scalar(d2, kv, xy[:, 1:2], None,
                                    op0=mybir.AluOpType.subtract)
            sq2 = pool.tile([128, KLO], F32)
            nc.vector.tensor_tensor(sq2, d2, d2, op=mybir.AluOpType.mult)

            wgt = pool.tile([128, KLO], F32)
            nc.vector.tensor_scalar(wgt, sq2, sq1[:, 0:1], None,
                                    op0=mybir.AluOpType.add)

            # --- out[w] = sum_klo st[:, klo, :].T @ wgt[:, klo] ---
            acc = psum_pool.tile([128, 1], F32)
            for klo in range(KLO):
                nc.tensor.matmul(acc[:, :], st[:, klo, :],
                                 wgt[:, klo:klo + 1],
                                 start=(klo == 0), stop=(klo == KLO - 1))

            outv = pool.tile([128, 1], F32)
            nc.vector.tensor_copy(outv, acc[:, :])
            nc.sync.dma_start(out=outd[:, :], in_=outv[:, :])

    return nc


def _get_bass():
    if "nc" not in _CACHE:
        nc = _build_bass()
        nc.finalize()
        _CACHE["nc"] = nc
    return _CACHE["nc"]


def _make_blob(jlo):
    blob = np.zeros((128, NCOLS), dtype=np.float32)
    p = np.arange(128)
    blob[:, COL_ID:COL_ID + 128] = np.eye(128, dtype=np.float32)
    blob[:, COL_IOTA] = p
    blob[:, COL_JV] = jlo + p // KHI
    blob[:, COL_KV:COL_KV + KLO] = ((p % KHI) * KLO)[:, None] + np.arange(KLO)
    return blob


def _make_in_maps(tensor):
    t0 = np.ascontiguousarray(tensor[0], dtype=np.float32)  # [C,H,W]
    in_maps = []
    for c in range(NCORES):
        jlo = c * JPER
        blob = _make_blob(jlo)
        blob[:, COL_MAP:COL_MAP + 128] = t0[0]
        in_maps.append({
            "blob": blob,
            "tslice": np.ascontiguousarray(
                t0[jlo:jlo + JPER].reshape(128, KLO * W)),
        })
    return in_maps


def kernel(tensor):
    nc = _get_bass()
    res = run_bass_kernel_spmd(nc, _make_in_maps(tensor),
                               core_ids=list(range(NCORES)))
    partials = np.stack([r["out"].reshape(W) for r in res.results])
    return partials.astype(np.float64).sum(axis=0).astype(np.float32)


# revision 12
# speedup vs baseline: 1.2991x; 1.2293x over previous
"""Trainium2 Bass kernel for nn_Loss_39341900431615.

Reference semantics (B,C,H,W = 16,128,128,128; only tensor[0] is read):
    idx = argmax(tensor[0,0].reshape(-1))        # row-major first max
    x0, y0 = idx // W, idx % W
    wgt[j,k] = (x0-j)^2 + (y0-k)^2               # [H,W] = [128,128]
    out[w] = sum_{j,k} wgt[j,k] * tensor[0,j,k,w]  # [W] = [128]

Sharding: the j axis (channel dim of tensor[0]) is split across 8 cores,
16 j-planes each (1 MB/core). The [128,128] argmax map tensor[0,0] is
replicated to every core (packed in the const blob) and each core
computes the argmax redundantly. Each core emits a [128] partial; the
host sums the 8 partials.

Layout: the per-core slice rides as a flat [128, 2048] tensor — SBUF
partition p holds source plane (j = p//8, k in [16*(p%8), 16*(p%8)+16))
as 8 KB contiguous rows (full-bandwidth DMA). The weighted reduction
V[p, w] = sum_klo wgt2[p, klo] * st[p, klo, w] runs as a 16-step
ping-pong scalar_tensor_tensor chain on the DVE (cheaper than 16 PE
matmuls: fp32 matmuls cost two LDWEIGHTS+MATMUL passes each). A single
PE matmul ones_col.T @ V then reduces partitions, producing the result
as a [1, 128] contiguous row so the output DMA is one 512 B descriptor
(a [128, 1] column DMA costs ~6 us in per-descriptor overhead).

Per-core device program:
  1. DMA in: blob [128, 274] (map | identity | iota | jv2 | kv2),
     slice [128, 2048].
  2. Row max + row argmax of the map via DVE max_with_indices.
  3. PE-transposes (against the identity): iota row, rowmax (f32),
     rowargmax (uint32, cast on the PSUM->SBUF copy). max_with_indices
     on the transposed rowmax gives gmax and x0 (first occurrence ==
     row-major semantics); y0 = sum_r (r==x0)*rowargmax[r] straight
     into xs via accum_out.
  4. Broadcast (x0,y0) to all partitions with a K=1 matmul against ones.
  5. wgt2[p,klo] = (jv2[p]-x0)^2 + (kv2[p,klo]-y0)^2 via DVE ops.
  6. 16-step DVE chain -> V; one PE matmul -> psum [1, 128].
  7. Copy PSUM -> SBUF row, DMA out [1, 128].
"""

import sys

for _p in ("/opt/trn_rl_repo", "/opt/pypackages"):
    if _p not in sys.path:
        sys.path.insert(0, _p)

import numpy as np

import concourse.bass as bass
from concourse import bacc
import concourse.tile as tile
from concourse import mybir
from concourse.bass_utils import run_bass_kernel_spmd

B, C, H, W = 16, 128, 128, 128
NCORES = 8
JPER = C // NCORES   # 16 j-planes per core
KLO = 16             # inner contraction steps (k per partition)
KHI = 8              # k groups per partition dim
F32 = mybir.dt.float32

# const blob column layout
COL_MAP = 0            # [128, 128] argmax map
COL_ID = 128           # [128, 128] identity
COL_IOTA = 256         # [128, 1] partition index 0..127
COL_JV = 257           # [128, 1] j(p) = jlo + p//8
COL_KV = 258           # [128, 16] k(p, klo) = (p%8)*16 + klo
NCOLS = 274

_CACHE = {}


def _build_bass():
    nc = bacc.Bacc("TRN2", target_bir_lowering=False, debug=False,
                   num_devices=NCORES, enable_partition_id=False)

    blob_d = nc.dram_tensor("blob", [128, NCOLS], F32, kind="ExternalInput")
    tslice = nc.dram_tensor("tslice", [128, KLO * W], F32,
                            kind="ExternalInput")
    outd = nc.dram_tensor("out", [1, W], F32, kind="ExternalOutput")

    with tile.TileContext(nc) as tc:
        with (
            tc.tile_pool(name="main", bufs=1) as pool,
            tc.tile_pool(name="psum", bufs=1, space="PSUM") as psum_pool,
        ):
            blob = pool.tile([128, NCOLS], F32)
            st = pool.tile([128, KLO, W], F32)     # [(j,khi), klo, w]

            nc.sync.dma_start(out=blob[:, :], in_=blob_d[:, :])
            nc.sync.dma_start(
                out=st[:, :, :],
                in_=tslice.ap().rearrange("p (a b) -> p a b", a=KLO))

            sm = blob[:, COL_MAP:COL_MAP + 128]    # argmax map [x, y]
            sid = blob[:, COL_ID:COL_ID + 128]     # identity
            si = blob[:, COL_IOTA:COL_IOTA + 1]    # partition index
            jv = blob[:, COL_JV:COL_JV + 1]        # j(p)
            kv = blob[:, COL_KV:COL_KV + KLO]      # k(p, klo)

            # --- per-row max and argmax of the map ---
            vmax8 = pool.tile([128, 8], F32)
            vidx8 = pool.tile([128, 8], mybir.dt.uint32)
            nc.vector.max_with_indices(vmax8, vidx8, sm)

            vidxf = pool.tile([128, 1], F32)       # rowargmax as f32
            nc.vector.tensor_copy(vidxf, vidx8[:, 0:1])

            # --- PE transposes (iota first: its only dep is the blob DMA)
            iotaT_ps = psum_pool.tile([1, 128], F32)
            nc.tensor.transpose(iotaT_ps[:, :], si, sid)
            vmaxT_ps = psum_pool.tile([1, 128], F32)
            nc.tensor.transpose(vmaxT_ps[:, :], vmax8[:, 0:1], sid)
            vidxT_ps = psum_pool.tile([1, 128], F32)
            nc.tensor.transpose(vidxT_ps[:, :], vidxf[:, :], sid)

            iotaw = pool.tile([1, 128], F32)
            nc.vector.tensor_copy(iotaw, iotaT_ps[:, :])
            vmaxT = pool.tile([1, 128], F32)
            nc.vector.tensor_copy(vmaxT, vmaxT_ps[:, :])
            vidxT = pool.tile([1, 128], F32)
            nc.vector.tensor_copy(vidxT, vidxT_ps[:, :])

            # global max over rows: value + first row index (= x0)
            gv8 = pool.tile([1, 8], F32)
            gi8 = pool.tile([1, 8], mybir.dt.uint32)
            nc.vector.max_with_indices(gv8, gi8, vmaxT[:, :])

            xs = pool.tile([1, 2], F32)            # (x0, y0) on partition 0
            nc.vector.tensor_copy(xs[:, 0:1], gi8[:, 0:1])

            # y0 = sum_r (r == x0) * rowargmax[r], straight into xs[:,1]
            ymask = pool.tile([1, 128], F32)
            nc.vector.scalar_tensor_tensor(
                ymask, in0=iotaw[:, :], scalar=xs[:, 0:1], in1=vidxT[:, :],
                op0=mybir.AluOpType.is_equal, op1=mybir.AluOpType.mult,
                accum_out=xs[:, 1:2],
            )

            # broadcast (x0, y0) to all partitions via K=1 matmul with ones
            ones = pool.tile([1, 128], F32)
            nc.vector.memset(ones, 1.0)
            onescol = pool.tile([128, 1], F32)
            nc.vector.memset(onescol, 1.0)
            xy_ps = psum_pool.tile([128, 2], F32)
            nc.tensor.matmul(xy_ps[:, :], ones[:, :], xs[:, :],
                             start=True, stop=True)
            xy = pool.tile([128, 2], F32)
            nc.vector.tensor_copy(xy, xy_ps[:, :])

            # --- wgt2[p, klo] = (jv[p]-x0)^2 + (kv[p,klo]-y0)^2 ---
            d1 = pool.tile([128, 1], F32)
            nc.vector.tensor_scalar(d1, jv, xy[:, 0:1], None,
                                    op0=mybir.AluOpType.subtract)
            sq1 = pool.tile([128, 1], F32)
            nc.vector.tensor_tensor(sq1, d1, d1, op=mybir.AluOpType.mult)

            d2 = pool.tile([128, KLO], F32)
            nc.vector.tensor_scalar(d2, kv, xy[:, 1:2], None,
                                    op0=mybir.AluOpType.subtract)
            sq2 = pool.tile([128, KLO], F32)
            nc.vector.tensor_tensor(sq2, d2, d2, op=mybir.AluOpType.mult)

            wgt = pool.tile([128, KLO], F32)
            nc.vector.tensor_scalar(wgt, sq2, sq1[:, 0:1], None,
                                    op0=mybir.AluOpType.add)

            # --- V[p, w] = sum_klo st[p, klo, :] * wgt[p, klo] ---
            # ping-pong scalar_tensor_tensor chain on the DVE
            va = pool.tile([128, W], F32)
            vb = pool.tile([128, W], F32)
            nc.vector.tensor_scalar(va, st[:, 0, :], wgt[:, 0:1], None,
                                    op0=mybir.AluOpType.mult)
            cur, nxt = va, vb
            for klo in range(1, KLO):
                nc.vector.scalar_tensor_tensor(
                    nxt, in0=st[:, klo, :], scalar=wgt[:, klo:klo + 1],
                    in1=cur,
                    op0=mybir.AluOpType.mult, op1=mybir.AluOpType.add)
                cur, nxt = nxt, cur

            # --- out[1, w] = ones_col.T @ V (partition reduce) ---
            out_ps = psum_pool.tile([1, W], F32)
            nc.tensor.matmul(out_ps[:, :], onescol[:, :], cur[:, :],
                             start=True, stop=True)

            outv = pool.tile([1, W], F32)
            nc.vector.tensor_copy(outv, out_ps[:, :])
            nc.sync.dma_start(out=outd[:, :], in_=outv[:, :])

    return nc


def _get_bass():
    if "nc" not in _CACHE:
        nc = _build_bass()
        nc.finalize()
        _CACHE["nc"] = nc
    return _CACHE["nc"]


def _make_blob(jlo):
    blob = np.zeros((128, NCOLS), dtype=np.float32)
    p = np.arange(128)
    blob[:, COL_ID:COL_ID + 128] = np.eye(128, dtype=np.float32)
    blob[:, COL_IOTA] = p
    blob[:, COL_JV] = jlo + p // KHI
    blob[:, COL_KV:COL_KV + KLO] = ((p % KHI) * KLO)[:, None] + np.arange(KLO)
    return blob


def _make_in_maps(tensor):
    t0 = np.ascontiguousarray(tensor[0], dtype=np.float32)  # [C,H,W]
    in_maps = []
    for c in range(NCORES):
        jlo = c * JPER
        blob = _make_blob(jlo)
        blob[:, COL_MAP:COL_MAP + 128] = t0[0]
        in_maps.append({
            "blob": blob,
            "tslice": np.ascontiguousarray(
                t0[jlo:jlo + JPER].reshape(128, KLO * W)),
        })
    return in_maps


def kernel(tensor):
    nc = _get_bass()
    res = run_bass_kernel_spmd(nc, _make_in_maps(tensor),
                               core_ids=list(range(NCORES)))
    partials = np.stack([r["out"].reshape(W) for r in res.results])
    return partials.astype(np.float64).sum(axis=0).astype(np.float32)
